# revision 1
# baseline (speedup 1.0000x reference)
"""Trainium2 Bass kernel for nn_MultiHeadAttention (B=2, S=2048, E=1024, H=16, D=64).

Sharding: 8 cores = 2 batches x 4 head-groups (4 heads / core, d_local=256).
Each core computes, for its (batch b, head group g):
    q = Xq[b] @ Wq[:, hs]*0.125 + bq[hs]*0.125        (transposed layout QT [256, S])
    k = Xk[b] @ Wk[:, hs] + bk[hs]                    (transposed layout KT [256, S])
    v = Xv[b] @ Wv[:, hs] + bv[hs]                    (natural layout, 65-strided + ones col)
    per head: scores^T = K_h @ Q_h^T  -> exp (ACT) -> Z|denom = expW^T.T @ [V_h|1]
    Z normalized per-partition, PE-transposed to ZT [256, S]
    partial_out = Z @ Wo[hs, :]                       ([S, E] fp32, host sums over g)
Host: transposes/casts inputs to bf16, sums the 4 partials per batch, adds bo.

Self-contained: hardcodes all shapes; requires only concourse (+ml_dtypes/numpy).
"""

import sys
import types

import numpy as np
import ml_dtypes

import concourse.bass as bass  # noqa: F401  (bass types used via tile/bacc)
import concourse.mybir as mybir
import concourse.tile as tile
from concourse import bacc
from concourse import bass_utils
from concourse.masks import make_identity

BF16 = mybir.dt.bfloat16
F32 = mybir.dt.float32
AF = mybir.ActivationFunctionType

B, S, E = 2, 2048, 1024
H, D = 16, 64
N_CORES = 8
HL = 4          # heads per core
DL = HL * D     # 256 local d
NPAIR = 2       # head pairs per core
KT_TILES = S // 128   # 16
QC = 4          # q chunks of 512
ET = E // 128   # 8 e-tiles


def _install_ntff_hook():
    """Register the axon NTFF profiling hook if the image's antenv lacks it."""
    try:
        import antenv  # noqa
        if 'antenv.axon_hooks' in sys.modules:
            return
        mod = types.ModuleType('antenv.axon_hooks')
        _hook = [None]
        mod.set_axon_ntff_profile_hook = lambda h: _hook.__setitem__(0, h)
        mod.get_axon_ntff_profile_hook = lambda: _hook[0]
        sys.modules['antenv.axon_hooks'] = mod
        setattr(antenv, 'axon_hooks', mod)
        try:
            from trn_agent_boot.trn_boot import _ntff_profile_via_ctypes
            h = _ntff_profile_via_ctypes('/opt/axon/libaxon_pjrt.so')
            if h is not None:
                mod.set_axon_ntff_profile_hook(h)
        except Exception:
            pass
    except Exception:
        pass


def build_kernel():
    nc = bacc.Bacc("TRN2", target_bir_lowering=False, debug=False,
                   enable_asserts=True, num_devices=N_CORES)

    # all inputs pre-arranged on host to be contiguous for their SBUF tiles
    xq_ap = nc.dram_tensor("xq_t", [QC, 128, ET, 512], BF16, kind="ExternalInput").ap()
    xk_ap = nc.dram_tensor("xk_t", [QC, 128, ET, 512], BF16, kind="ExternalInput").ap()
    xv_ap = nc.dram_tensor("xv_t", [QC, 128, ET, 512], BF16, kind="ExternalInput").ap()
    wq_ap = nc.dram_tensor("wq", [128, 2, ET, 128], BF16, kind="ExternalInput").ap()
    wk_ap = nc.dram_tensor("wk", [128, 2, ET, 128], BF16, kind="ExternalInput").ap()
    wv_ap = nc.dram_tensor("wv", [128, ET, HL * 65], BF16, kind="ExternalInput").ap()
    bq_ap = nc.dram_tensor("bq", [128, 2], F32, kind="ExternalInput").ap()
    bk_ap = nc.dram_tensor("bk", [128, 2], F32, kind="ExternalInput").ap()
    bv_ap = nc.dram_tensor("bv", [1, HL * 65], BF16, kind="ExternalInput").ap()
    wo_ap = nc.dram_tensor("wo", [128, 2, E], BF16, kind="ExternalInput").ap()
    out_ap = nc.dram_tensor("out_p", [S, E], BF16, kind="ExternalOutput").ap()

    from contextlib import ExitStack
    with tile.TileContext(nc) as tc, ExitStack() as ctx:
        wpool = ctx.enter_context(tc.tile_pool(name="w", bufs=1))
        xtp = ctx.enter_context(tc.tile_pool(name="xt", bufs=5))
        big = ctx.enter_context(tc.tile_pool(name="big", bufs=1))
        expp = ctx.enter_context(tc.tile_pool(name="expp", bufs=4))
        znp = ctx.enter_context(tc.tile_pool(name="znp", bufs=2))
        smal = ctx.enter_context(tc.tile_pool(name="small", bufs=2))
        stg = ctx.enter_context(tc.tile_pool(name="stg", bufs=4))
        pscore = ctx.enter_context(tc.tile_pool(name="pscore", bufs=2, space="PSUM"))
        pav = ctx.enter_context(tc.tile_pool(name="pav", bufs=1, space="PSUM"))
        ptr = ctx.enter_context(tc.tile_pool(name="ptr", bufs=1, space="PSUM"))
        ppo = ctx.enter_context(tc.tile_pool(name="ppo", bufs=2, space="PSUM"))

        # ---- persistent weights / constants ----
        wq_sb = wpool.tile([128, 2, ET, 128], BF16, tag="wq")
        wk_sb = wpool.tile([128, 2, ET, 128], BF16, tag="wk")
        wv_sb = wpool.tile([128, ET, HL * 65], BF16, tag="wv")
        wo_sb = wpool.tile([128, 2, E], BF16, tag="wo")
        bq_sb = wpool.tile([128, 2], F32, tag="bq")
        bk_sb = wpool.tile([128, 2], F32, tag="bk")
        bv_sb = wpool.tile([1, HL * 65], BF16, tag="bv")
        ones_col = wpool.tile([1, 128], BF16, tag="ones")
        ident = wpool.tile([128, 128], BF16, tag="ident")

        nc.vector.memset(ones_col[:], 1.0)
        make_identity(nc, ident[:])

        QT = big.tile([128, NPAIR, S], BF16, tag="QT")
        KT = big.tile([128, NPAIR, S], BF16, tag="KT")
        Vones = big.tile([128, KT_TILES, HL, 65], BF16, tag="Vones")
        ZT = big.tile([128, NPAIR, S], BF16, tag="ZT")

        def load_xt_half(ap, sc, hf, eng=None):
            # half of a 512-column slice of X^T: [128, 4 e-tiles, 512]
            t = xtp.tile([128, ET // 2, 512], BF16, tag="xt", name="xt")
            (eng or nc.sync).dma_start(t[:], ap[sc][:, hf * 4:(hf + 1) * 4, :])
            return t

        def load_xt_sc(ap, sc, split_rings=False):
            # split_rings: h1 rides the Activation HWDGE ring in parallel.
            # Only safe for fresh tiles with no WAR (head loads) - a waiting
            # DMA on the ACT ring would block queued EXPs (strict FIFO).
            return (load_xt_half(ap, sc, 0),
                    load_xt_half(ap, sc, 1, nc.scalar if split_rings else None))

        def load_xt_half_fine(ap, sc, hf):
            # same tile, but two 2-et DMA chunks -> 2 queues in parallel
            t = xtp.tile([128, ET // 2, 512], BF16, tag="xt", name="xt")
            for c in range(2):
                nc.sync.dma_start(
                    t[:, 2 * c:2 * c + 2, :],
                    ap[sc][:, hf * 4 + 2 * c:hf * 4 + 2 * c + 2, :])
            return t

        def load_xt_sc_fine(ap, sc):
            return (load_xt_half_fine(ap, sc, 0), load_xt_half_fine(ap, sc, 1))

        def xe(x_pair, e):
            return x_pair[e // 4][:, e % 4, :]

        def proj_qk_sc(dst, w_sb, b_sb, x_sc, p, sc):
            # dst[:, p, sc-block] (transposed proj): out[d(128), s] = W.T @ X^T
            ps = ppo.tile([128, 512], F32, tag="ppo")
            for e in range(ET):
                nc.tensor.matmul(
                    ps[:], w_sb[:, p, e, :], xe(x_sc, e),
                    start=(e == 0), stop=(e == ET - 1))
            nc.vector.tensor_scalar_add(
                dst[:, p, sc * 512:(sc + 1) * 512], ps[:], b_sb[:, p:p + 1])

        def gen_projqk(dst, w_sb, b_sb, x_ap, p, split_first=False):
            # generator: ~2 matmuls (0.45us) per pull; first pull per sc just
            # issues the x DMAs (so co-drained generators overlap their loads)
            for sc in range(QC):
                x_sc = load_xt_sc(x_ap, sc, split_rings=(split_first and sc == 0))
                yield
                ps = ppo.tile([128, 512], F32, tag="ppo", name="ps")
                for e in range(ET):
                    nc.tensor.matmul(
                        ps[:], w_sb[:, p, e, :], xe(x_sc, e),
                        start=(e == 0), stop=(e == ET - 1))
                    if e % 2 == 1:
                        yield
                nc.vector.tensor_scalar_add(
                    dst[:, p, sc * 512:(sc + 1) * 512], ps[:], b_sb[:, p:p + 1])
                yield

        def gen_projv(x_ap):
            # single-pass V projection (all 4 heads, N=260), smeared
            for vsc in range(QC):
                x_sc = load_xt_sc(x_ap, vsc)
                for sti in range(4):
                    st = vsc * 4 + sti
                    ps = ppo.tile([128, HL * 65], F32, tag="ppo", name="ps")
                    for e in range(ET):
                        nc.tensor.matmul(
                            ps[:], xe(x_sc, e)[:, sti * 128:(sti + 1) * 128],
                            wv_sb[:, e, :],
                            start=(e == 0), stop=False)
                        if e % 2 == 1:
                            yield
                    nc.tensor.matmul(ps[:], ones_col[:], bv_sb[:],
                                     start=False, stop=True)
                    nc.vector.tensor_copy(
                        Vones[:, st], ps[:].rearrange("p (h d) -> p h d", h=HL))
                    yield

        def gen_av(p, qc, et):
            # AV + normalize + transpose for one (pair, q-chunk); ~0.3us per pull
            zn = znp.tile([128, 4, 2, D], BF16, tag="zn", name="zn")
            for h in range(2):
                avp = pav.tile([128, 4, 65], F32, tag="av", name="avp")
                # qt-outer: interleaved accumulation groups in one PSUM bank
                # are NOT allowed (each group's start clears the whole bank's
                # has_written bits) — a qt group must fully precede the next.
                for qt in range(4):
                    for kt in range(KT_TILES):
                        nc.tensor.matmul(
                            avp[:, qt, :],
                            et[:, kt, h, qt * 128:(qt + 1) * 128],
                            Vones[:, kt, 2 * p + h, :],
                            start=(kt == 0), stop=(kt == KT_TILES - 1))
                        if kt % 8 == 7:
                            yield
                rc = smal.tile([128, 4, 1], F32, tag="rc", name="rc")
                nc.vector.reciprocal(rc[:], avp[:, :, 64:65])
                nc.vector.tensor_mul(zn[:, :, h, :], avp[:, :, 0:D],
                                     rc[:].to_broadcast([128, 4, D]))
                yield
            for qt in range(4):
                tp = ptr.tile([128, 128], BF16, tag="tr", name="tp")
                nc.tensor.transpose(tp[:], zn[:, qt], ident[:])
                nc.vector.tensor_copy(
                    ZT[:, p, qc * 512 + qt * 128: qc * 512 + (qt + 1) * 128], tp[:])
                if qt % 2 == 1:
                    yield

        def gen_outproj(sts, act_evict=False, split_dma=False):
            for st in sts:
                stt = stg.tile([128, 2, 512], BF16, tag="stg", name="stt")
                for ec in range(2):
                    ps = ppo.tile([128, 512], F32, tag="ppo", name="ps")
                    for dt_ in range(2):
                        nc.tensor.matmul(
                            ps[:], ZT[:, dt_, st * 128:(st + 1) * 128],
                            wo_sb[:, dt_, ec * 512:(ec + 1) * 512],
                            start=(dt_ == 0), stop=(dt_ == 1))
                    if act_evict and ec == 1:
                        nc.scalar.copy(stt[:, ec], ps[:])
                    else:
                        nc.vector.tensor_copy(stt[:, ec], ps[:])
                    rows = out_ap[st * 128:(st + 1) * 128]
                    # last two st-units: ec1 DMA rides the idle scalar ring so
                    # the final two output chunks drain in parallel
                    ring = nc.scalar if (act_evict and st >= 14 and ec == 1) \
                        else nc.sync
                    ring.dma_start(rows[:, ec * 512:(ec + 1) * 512],
                                   stt[:, ec])
                    yield

        def scores_kts(p, qc, et, kts):
            for kt in kts:
                sc_t = pscore.tile([128, 2, 512], F32, tag="sc")
                for h in range(2):
                    nc.tensor.matmul(
                        sc_t[:, h, :],
                        KT[64 * h:64 * (h + 1), p, kt * 128:(kt + 1) * 128],
                        QT[64 * h:64 * (h + 1), p, qc * 512:(qc + 1) * 512],
                        start=True, stop=True, tile_position=(64 * h, 0))
                nc.scalar.activation(et[:, kt], sc_t[:], AF.Exp)

        def new_et():
            return expp.tile([128, KT_TILES, 2, 512], BF16, tag="expT", name="et")

        def drain(g, n=10 ** 9):
            """Pull generator g up to n times; True if exhausted."""
            for _ in range(n):
                if next(g, StopIteration) is StopIteration:
                    return True
            return False

        # ---- emission (static per-engine order ~ schedule priority) ----
        # PE warmup (HAM): dummy matmuls on a zeroed tile during the DMA lead-in
        warm = wpool.tile([128, 256], BF16, tag="warm")
        nc.vector.memset(warm[:], 0.0)
        wps = ppo.tile([128, 512], F32, tag="ppo")
        for i in range(26):
            nc.tensor.matmul(wps[:, 0:256], warm[:, 0:128], warm[:],
                             start=(i == 0), stop=(i == 25))

        # DMA rings: weights + outputs ride the Activation (scalar) HWDGE ring,
        # x inputs ride the SP (sync) ring -> the two streams run in parallel
        # (head: wq/wk no longer serialize behind xq/xk sc0)
        nc.scalar.dma_start(wq_sb[:, 0], wq_ap[:, 0])
        nc.scalar.dma_start(wk_sb[:, 0], wk_ap[:, 0])

        # first q-chunk of scores interleaved with the pair-0 QK projections
        # (generator-based so PE stays fed between scores bursts while ACT
        # drains the exp backlog)
        nc.scalar.dma_start(bq_sb[:], bq_ap[:])
        nc.scalar.dma_start(bk_sb[:], bk_ap[:])
        et00 = expp.tile([128, KT_TILES, 2, 512], BF16, tag="expT")
        gqt0 = gen_projqk(QT, wq_sb, bq_sb, xq_ap, 0, split_first=True)
        gkt0 = gen_projqk(KT, wk_sb, bk_sb, xk_ap, 0, split_first=True)
        drain(gqt0, 1)   # issue xq-sc0 DMAs
        drain(gkt0, 1)   # issue xk-sc0 DMAs (parallel on the scalar ring)
        drain(gqt0, 5)   # QT0 qc0 proj + evict
        for b in range(4):
            drain(gkt0, 5 if b == 0 else 6)   # KT0 k-range for kt 4b..4b+3
            scores_kts(0, 0, et00, [4 * b, 4 * b + 1])
            drain(gqt0, 3)
            scores_kts(0, 0, et00, [4 * b + 2, 4 * b + 3])
            drain(gqt0, 2)
        drain(gqt0)
        drain(gkt0)

        # pair-1 weights (for gqt1/gkt1) + V path loads
        nc.scalar.dma_start(wq_sb[:, 1], wq_ap[:, 1])
        nc.scalar.dma_start(wk_sb[:, 1], wk_ap[:, 1])
        nc.scalar.dma_start(wv_sb[:], wv_ap[:])
        nc.scalar.dma_start(bv_sb[:], bv_ap[:])
        nc.scalar.dma_start(wo_sb[:], wo_ap[:])

        # background work generators, smeared between scores kt's (FIFO so only
        # one AV psum tile is live at a time)
        gv = gen_projv(xv_ap)
        gqt1 = gen_projqk(QT, wq_sb, bq_sb, xq_ap, 1)
        gkt1 = gen_projqk(KT, wk_sb, bk_sb, xk_ap, 1)
        ets = {(0, 0): et00}

        def run_loop(p, qc, fifo, budget):
            """Emit scores(p, qc) kt-by-kt, pulling `budget` steps per kt from
            the FIFO of filler generators."""
            et = ets.setdefault((p, qc), new_et())
            for kt in range(KT_TILES):
                scores_kts(p, qc, et, [kt])
                left = budget
                while left > 0 and fifo:
                    if drain(fifo[0], left):
                        fifo.pop(0)
                        left -= 1  # approximate
                    else:
                        left = 0

        # block-style emission: scores 4-kt blocks alternating with background
        # work blocks (~one vsc/sc group at a time)
        # loop 0: scores(0,1) + first half of V projection (2-kt granularity)
        et01 = ets.setdefault((0, 1), new_et())
        for i in range(8):
            scores_kts(0, 1, et01, range(2 * i, 2 * i + 2))
            drain(gv, 5)
        # loop 1: scores(0,2) + rest of V
        et02 = ets.setdefault((0, 2), new_et())
        for i in range(8):
            scores_kts(0, 2, et02, range(2 * i, 2 * i + 2))
            drain(gv, 5)
        drain(gv)
        # loop 2: scores(0,3) + QT1 + av(0,0)
        ga00 = gen_av(0, 0, ets[(0, 0)])
        et03 = ets.setdefault((0, 3), new_et())
        for i in range(8):
            scores_kts(0, 3, et03, range(2 * i, 2 * i + 2))
            drain(gqt1, 3)
            drain(ga00, 3)
        drain(gqt1)
        drain(ga00)
        # loop 3: scores(1,0) + KT1 (sc-block ordered ahead) + av(0,1)
        ga01 = gen_av(0, 1, ets[(0, 1)])
        et10 = ets.setdefault((1, 0), new_et())
        for sc in range(QC):
            drain(gkt1, 6)  # one full sc block of KT1
            scores_kts(1, 0, et10, range(4 * sc, 4 * sc + 2))
            drain(ga01, 3)
            scores_kts(1, 0, et10, range(4 * sc + 2, 4 * sc + 4))
            drain(ga01, 3)
        drain(gkt1)
        drain(ga01)
        # loop 4: scores(1,1) + av(1,0) + av(0,2)
        ga10 = gen_av(1, 0, ets[(1, 0)])
        ga02 = gen_av(0, 2, ets[(0, 2)])
        et11 = ets.setdefault((1, 1), new_et())
        gop0 = gen_outproj(range(0, 4))
        for i in range(8):
            scores_kts(1, 1, et11, range(2 * i, 2 * i + 2))
            drain(ga10, 3)
            drain(ga02, 3)
            if i >= 7:
                drain(gop0, 2)
        drain(ga10)
        drain(ga02)
        # loop 5: scores(1,2) + av(1,1) + av(0,3) + outproj(0-3)
        ga11 = gen_av(1, 1, ets[(1, 1)])
        ga03 = gen_av(0, 3, ets[(0, 3)])
        et12 = ets.setdefault((1, 2), new_et())
        for i in range(8):
            scores_kts(1, 2, et12, range(2 * i, 2 * i + 2))
            drain(ga11, 3)
            drain(ga03, 3)
            drain(gop0, 1)
        drain(ga11)
        drain(ga03)
        # loop 6: scores(1,3) + av(1,2) + outproj(4-11)
        ga12 = gen_av(1, 2, ets[(1, 2)])
        gop1 = gen_outproj(range(4, 8))
        gop2 = gen_outproj(range(8, 12))
        et13 = ets.setdefault((1, 3), new_et())
        for i in range(8):
            scores_kts(1, 3, et13, range(2 * i, 2 * i + 2))
            drain(gop0, 1)
            drain(ga12, 4)   # 20 yields total -> exhausted by i=4
            drain(gop1, 1)
            if i >= 5:
                # ga12 (incl. its ZT transposes) fully drained, so outproj
                # st8-11 (which reads ZT qc2) is emitted safely after
                drain(gop2, 3)
        drain(gop0)
        drain(ga12)
        drain(gop1)
        drain(gop2)
        # tail: av(1,3) interleaved with outproj(12-15) at qt granularity
        # (gen_av yields 20x: 9 per h-loop, then transposes at 19 & 20)
        ga13 = gen_av(1, 3, ets[(1, 3)])
        gop3 = gen_outproj(range(12, 16), act_evict=True)
        drain(ga13, 19)    # h-loops + qt0/qt1 transposes
        drain(gop3, 4)     # st12, st13
        drain(ga13)        # qt2/qt3 transposes
        drain(gop3)        # st14, st15

    nc.compile()
    return nc


def prep_inputs(query, key, value, Wq, bq, Wk, bk, Wv, bv, Wo, bo):
    """Host-side sharding: per-core input dicts (bf16, transposed/augmented)."""
    bf = ml_dtypes.bfloat16
    q32 = np.asarray(query, np.float32)
    k32 = np.asarray(key, np.float32)
    v32 = np.asarray(value, np.float32)
    Wq = np.asarray(Wq, np.float32)
    Wk = np.asarray(Wk, np.float32)
    Wv = np.asarray(Wv, np.float32)
    Wo = np.asarray(Wo, np.float32)
    bq = np.asarray(bq, np.float32)
    bk = np.asarray(bk, np.float32)
    bv = np.asarray(bv, np.float32)

    scale = 1.0 / np.sqrt(np.float32(D))

    def xt_layout(x2d):
        # [S, E] -> X^T [E, S] -> [sc, p, eo, j] contiguous tile layout
        a = x2d.T.reshape(ET, 128, QC, 512).transpose(2, 1, 0, 3)
        return np.ascontiguousarray(a).astype(bf)

    def w_layout(w2d):
        # [E, D'] -> [p, eo, D'] contiguous
        a = w2d.reshape(ET, 128, w2d.shape[1]).transpose(1, 0, 2)
        return np.ascontiguousarray(a).astype(bf)

    def w_layout_pair(w2d):
        # [E, 256] -> [p, pair, eo, 128] contiguous (pair-major so pair-0
        # loads alone in the head)
        a = w2d.reshape(ET, 128, 2, 128).transpose(1, 2, 0, 3)
        return np.ascontiguousarray(a).astype(bf)

    xt = {}
    for b in range(B):
        xt[('q', b)] = xt_layout(q32[b])
        xt[('k', b)] = xt_layout(k32[b])
        xt[('v', b)] = xt_layout(v32[b])

    in_maps = []
    for c in range(N_CORES):
        b, g = c // HL, c % HL
        hs = slice(g * DL, (g + 1) * DL)
        wv_aug = np.zeros((E, HL * 65), np.float32)
        bv_aug = np.zeros((1, HL * 65), np.float32)
        for h in range(HL):
            wv_aug[:, h * 65:h * 65 + D] = Wv[:, g * DL + h * D: g * DL + (h + 1) * D]
            bv_aug[0, h * 65:h * 65 + D] = bv[g * DL + h * D: g * DL + (h + 1) * D]
            bv_aug[0, h * 65 + D] = 1.0
        in_maps.append({
            "xq_t": xt[('q', b)],
            "xk_t": xt[('k', b)],
            "xv_t": xt[('v', b)],
            "wq": w_layout_pair(Wq[:, hs] * scale),
            "wk": w_layout_pair(Wk[:, hs]),
            "wv": w_layout(wv_aug),
            "bq": np.ascontiguousarray(
                (bq[hs] * scale).reshape(2, 128).T).astype(np.float32),
            "bk": np.ascontiguousarray(
                bk[hs].reshape(2, 128).T).astype(np.float32),
            "bv": bv_aug.astype(bf),
            "wo": np.ascontiguousarray(
                Wo[hs, :].reshape(2, 128, E).transpose(1, 0, 2)).astype(bf),
        })
    return in_maps


_NC_CACHE = [None]


def get_nc():
    if _NC_CACHE[0] is None:
        _install_ntff_hook()
        _NC_CACHE[0] = build_kernel()
    return _NC_CACHE[0]


def run(inputs, trace=False):
    nc = get_nc()
    in_maps = prep_inputs(**{k: v for k, v in inputs.items() if k != 'bo'},
                          bo=inputs['bo'])
    res = bass_utils.run_bass_kernel_spmd(
        nc, in_maps, core_ids=list(range(N_CORES)), trace=trace)
    bo = np.asarray(inputs['bo'], np.float32)
    out = np.empty((B, S, E), np.float32)
    for b in range(B):
        acc = np.zeros((S, E), np.float32)
        for g in range(HL):
            acc += np.asarray(res.results[b * HL + g]["out_p"], np.float32)
        out[b] = acc + bo[None, :]
    return out, res


def kernel(**inputs):
    out, _ = run(inputs, trace=False)
    return out



# revision 20
# speedup vs baseline: 1.0752x; 1.0752x over previous
"""Trainium2 Bass kernel for nn_MultiHeadAttention (B=2, S=2048, E=1024, H=16, D=64).

Sharding: 8 cores = 2 batches x 4 head-groups (4 heads / core, d_local=256).
Each core computes, for its (batch b, head group g):
    q = Xq[b] @ Wq[:, hs]*0.125 + bq[hs]*0.125        (transposed layout QT [256, S])
    k = Xk[b] @ Wk[:, hs] + bk[hs]                    (transposed layout KT [256, S])
    v = Xv[b] @ Wv[:, hs] + bv[hs]                    (natural layout, 65-strided + ones col)
    per head: scores^T = K_h @ Q_h^T  -> exp (ACT) -> Z|denom = expW^T.T @ [V_h|1]
    Z normalized per-partition, PE-transposed to ZT [256, S]
    partial_out = Z @ Wo[hs, :]                       ([S, E] fp32, host sums over g)
Host: transposes/casts inputs to bf16, sums the 4 partials per batch, adds bo.

Schedule: the exp stream (128 score-tile exps on ACT, ~1.15us each) is the
critical resource; emission is an exp-paced "backbone" of 8 groups of 16 kt
slots, (pair, qc) interleaved (0,0),(1,0),(0,1),(1,1),... with all other PE
work (QKV projections both pairs per x load, AV, transposes, out-proj) pulled
between slots from a FIFO of generators plus explicit dependency guards.

Self-contained: hardcodes all shapes; requires only concourse (+ml_dtypes/numpy).
"""

import sys
import types

import numpy as np
import ml_dtypes

import concourse.bass as bass  # noqa: F401  (bass types used via tile/bacc)
import concourse.mybir as mybir
import concourse.tile as tile
from concourse import bacc
from concourse import bass_utils
from concourse.masks import make_identity

BF16 = mybir.dt.bfloat16
F32 = mybir.dt.float32
AF = mybir.ActivationFunctionType

B, S, E = 2, 2048, 1024
H, D = 16, 64
N_CORES = 8
HL = 4          # heads per core
DL = HL * D     # 256 local d
NPAIR = 2       # head pairs per core
KT_TILES = S // 128   # 16
QC = 4          # q chunks of 512
ET = E // 128   # 8 e-tiles


def _install_ntff_hook():
    """Register the axon NTFF profiling hook if the image's antenv lacks it."""
    try:
        import antenv  # noqa
        if 'antenv.axon_hooks' in sys.modules:
            return
        mod = types.ModuleType('antenv.axon_hooks')
        _hook = [None]
        mod.set_axon_ntff_profile_hook = lambda h: _hook.__setitem__(0, h)
        mod.get_axon_ntff_profile_hook = lambda: _hook[0]
        sys.modules['antenv.axon_hooks'] = mod
        setattr(antenv, 'axon_hooks', mod)
        try:
            from trn_agent_boot.trn_boot import _ntff_profile_via_ctypes
            h = _ntff_profile_via_ctypes('/opt/axon/libaxon_pjrt.so')
            if h is not None:
                mod.set_axon_ntff_profile_hook(h)
        except Exception:
            pass
    except Exception:
        pass


def build_kernel():
    nc = bacc.Bacc("TRN2", target_bir_lowering=False, debug=False,
                   enable_asserts=True, num_devices=N_CORES)

    # all inputs pre-arranged on host to be contiguous for their SBUF tiles
    xq_ap = nc.dram_tensor("xq_t", [QC, 128, ET, 512], BF16, kind="ExternalInput").ap()
    xk_ap = nc.dram_tensor("xk_t", [QC, 128, ET, 512], BF16, kind="ExternalInput").ap()
    xv_ap = nc.dram_tensor("xv_t", [QC, 128, ET, 512], BF16, kind="ExternalInput").ap()
    wq_ap = nc.dram_tensor("wq", [128, 2, ET, 128], BF16, kind="ExternalInput").ap()
    wk_ap = nc.dram_tensor("wk", [128, 2, ET, 128], BF16, kind="ExternalInput").ap()
    wv_ap = nc.dram_tensor("wv", [128, ET, HL * 65], BF16, kind="ExternalInput").ap()
    bq_ap = nc.dram_tensor("bq", [128, 2], F32, kind="ExternalInput").ap()
    bk_ap = nc.dram_tensor("bk", [128, 2], F32, kind="ExternalInput").ap()
    bvf_ap = nc.dram_tensor("bvf", [128, HL * 65], BF16, kind="ExternalInput").ap()
    wo_ap = nc.dram_tensor("wo", [128, 2, E], BF16, kind="ExternalInput").ap()
    out_ap = nc.dram_tensor("out_p", [S, E], BF16, kind="ExternalOutput").ap()

    from contextlib import ExitStack
    with tile.TileContext(nc) as tc, ExitStack() as ctx:
        wpool = ctx.enter_context(tc.tile_pool(name="w", bufs=1))
        xtp = ctx.enter_context(tc.tile_pool(name="xt", bufs=12))
        big = ctx.enter_context(tc.tile_pool(name="big", bufs=1))
        expp = ctx.enter_context(tc.tile_pool(name="expp", bufs=3))
        znp = ctx.enter_context(tc.tile_pool(name="znp", bufs=2))
        smal = ctx.enter_context(tc.tile_pool(name="small", bufs=2))
        stg = ctx.enter_context(tc.tile_pool(name="stg", bufs=4))
        pscore = ctx.enter_context(tc.tile_pool(name="pscore", bufs=2, space="PSUM"))
        pav = ctx.enter_context(tc.tile_pool(name="pav", bufs=1, space="PSUM"))
        ptr = ctx.enter_context(tc.tile_pool(name="ptr", bufs=1, space="PSUM"))
        ppo = ctx.enter_context(tc.tile_pool(name="ppo", bufs=2, space="PSUM"))

        # ---- persistent weights / constants ----
        wq_sb = wpool.tile([128, 2, ET, 128], BF16, tag="wq")
        wk_sb = wpool.tile([128, 2, ET, 128], BF16, tag="wk")
        wv_sb = wpool.tile([128, ET, HL * 65], BF16, tag="wv")
        wo_sb = wpool.tile([128, 2, E], BF16, tag="wo")
        bq_sb = wpool.tile([128, 2], F32, tag="bq")
        bk_sb = wpool.tile([128, 2], F32, tag="bk")
        bvf_sb = wpool.tile([128, HL * 65], BF16, tag="bvf")
        ident = wpool.tile([128, 128], BF16, tag="ident")

        make_identity(nc, ident[:])

        QT = big.tile([128, NPAIR, S], BF16, tag="QT")
        KT = big.tile([128, NPAIR, S], BF16, tag="KT")
        Vones = big.tile([128, KT_TILES, HL, 65], BF16, tag="Vones")
        ZT = big.tile([128, NPAIR, S], BF16, tag="ZT")

        def load_xt_half(ap, sc, hf, chunks=1):
            # chunks>1: split across DMA queues for more head bandwidth
            t = xtp.tile([128, ET // 2, 512], BF16, tag="xt", name="xt")
            step = 4 // chunks
            for c in range(chunks):
                nc.sync.dma_start(
                    t[:, c * step:(c + 1) * step, :],
                    ap[sc][:, hf * 4 + c * step:hf * 4 + (c + 1) * step, :])
            return t

        def load_xt_sc(ap, sc, chunks=1):
            return (load_xt_half(ap, sc, 0, chunks),
                    load_xt_half(ap, sc, 1, chunks))

        def xe(x_pair, e):
            return x_pair[e // 4][:, e % 4, :]

        # prefetch caches: sc -> (half0, half1)
        xq_c, xk_c, xv_c = {}, {}, {}

        def prefetch(cache, ap, sc, chunks=1):
            if sc not in cache:
                cache[sc] = load_xt_sc(ap, sc, chunks)

        def take(cache, ap, sc):
            prefetch(cache, ap, sc)
            return cache.pop(sc)

        # Q/K projections as ONE sequential generator (so at most one live
        # ppo tile is held across yields), ordered to match x DMA arrival:
        # q0 first (unlocks the backbone), then all of xk, then q1-q3.
        # part 'a' = [q0, k0-3] feeds the early backbone; part 'b' = [q1-3]
        # sits AFTER gv in the FIFO so xv DMAs are issued before xq1-3.
        QK_SEQ_A = [('q', 0), ('k', 0), ('k', 1), ('k', 2), ('k', 3)]
        QK_SEQ_B = [('q', 1), ('q', 2), ('q', 3)]
        prog = {'a': 0, 'b': 0}   # completed (seq-pos, pair) units per part

        def need_qt(p, qc):
            if qc == 0:
                return 'a', p + 1
            return 'b', 2 * (qc - 1) + p + 1

        def need_kt(p, sc):
            return 'a', 2 * (1 + sc) + p + 1

        def gen_projqk(key, seq):
            # both pairs per x chunk (single-pass over x); ATOMIC p-groups
            # (no yield while a ppo tile is live) so guard-driven pulls can
            # never interleave another ppo user onto a live buffer.
            units = 0
            for which, sc in seq:
                if which == 'q':
                    cache, x_ap, dst, w_sb, b_sb = xq_c, xq_ap, QT, wq_sb, bq_sb
                else:
                    cache, x_ap, dst, w_sb, b_sb = xk_c, xk_ap, KT, wk_sb, bk_sb
                x_sc = take(cache, x_ap, sc)
                for p in range(2):
                    ps = ppo.tile([128, 512], F32, tag="ppo", name="ps")
                    for e in range(ET):
                        nc.tensor.matmul(
                            ps[:], w_sb[:, p, e, :], xe(x_sc, e),
                            start=(e == 0), stop=(e == ET - 1))
                    nc.vector.tensor_scalar_add(
                        dst[:, p, sc * 512:(sc + 1) * 512], ps[:], b_sb[:, p:p + 1])
                    units += 1
                    prog[key] = units
                    yield

        def gen_projv():
            # single-pass V projection (all 4 heads, N=260); bias added on
            # DVE. ATOMIC st-units (no yield while the ppo tile is live).
            for vsc in range(QC):
                x_sc = take(xv_c, xv_ap, vsc)
                if vsc + 1 < QC:
                    prefetch(xv_c, xv_ap, vsc + 1)
                yield
                for sti in range(4):
                    st = vsc * 4 + sti
                    ps = ppo.tile([128, HL * 65], F32, tag="ppo", name="ps")
                    for e in range(ET):
                        nc.tensor.matmul(
                            ps[:], xe(x_sc, e)[:, sti * 128:(sti + 1) * 128],
                            wv_sb[:, e, :],
                            start=(e == 0), stop=(e == ET - 1))
                    nc.vector.tensor_add(
                        Vones[:, st],
                        ps[:].rearrange("p (h d) -> p h d", h=HL),
                        bvf_sb[:].rearrange("p (h d) -> p h d", h=HL))
                    yield

        def gen_av(p, qc, et):
            # AV + normalize + transpose for one (pair, q-chunk)
            zn = znp.tile([128, 4, 2, D], BF16, tag="zn", name="zn")
            for h in range(2):
                avp = pav.tile([128, 4, 65], F32, tag="av", name="avp")
                # qt-outer: interleaved accumulation groups in one PSUM bank
                # are NOT allowed (each group's start clears the whole bank's
                # has_written bits) — a qt group must fully precede the next.
                for qt in range(4):
                    for kt in range(KT_TILES):
                        nc.tensor.matmul(
                            avp[:, qt, :],
                            et[:, kt, h, qt * 128:(qt + 1) * 128],
                            Vones[:, kt, 2 * p + h, :],
                            start=(kt == 0), stop=(kt == KT_TILES - 1))
                        if kt % 8 == 7:
                            yield
                rc = smal.tile([128, 4, 1], F32, tag="rc", name="rc")
                nc.vector.reciprocal(rc[:], avp[:, :, 64:65])
                nc.vector.tensor_mul(zn[:, :, h, :], avp[:, :, 0:D],
                                     rc[:].to_broadcast([128, 4, D]))
                yield
            for qt in range(4):
                tp = ptr.tile([128, 128], BF16, tag="tr", name="tp")
                nc.tensor.transpose(tp[:], zn[:, qt], ident[:])
                nc.vector.tensor_copy(
                    ZT[:, p, qc * 512 + qt * 128: qc * 512 + (qt + 1) * 128], tp[:])
                if qt % 2 == 1:
                    yield

        def gen_outproj(sts, act_evict=False):
            for st in sts:
                stt = stg.tile([128, 2, 512], BF16, tag="stg", name="stt")
                for ec in range(2):
                    ps = ppo.tile([128, 512], F32, tag="ppo", name="ps")
                    for dt_ in range(2):
                        nc.tensor.matmul(
                            ps[:], ZT[:, dt_, st * 128:(st + 1) * 128],
                            wo_sb[:, dt_, ec * 512:(ec + 1) * 512],
                            start=(dt_ == 0), stop=(dt_ == 1))
                    if act_evict and ec == 1:
                        nc.scalar.copy(stt[:, ec], ps[:])
                    else:
                        nc.vector.tensor_copy(stt[:, ec], ps[:])
                    rows = out_ap[st * 128:(st + 1) * 128]
                    # last two st-units: ec1 DMA rides the idle scalar ring so
                    # the final two output chunks drain in parallel
                    ring = nc.scalar if (act_evict and st >= 14 and ec == 1) \
                        else nc.sync
                    ring.dma_start(rows[:, ec * 512:(ec + 1) * 512],
                                   stt[:, ec])
                    yield

        def scores_kts(p, qc, et, kts):
            for kt in kts:
                sc_t = pscore.tile([128, 2, 512], F32, tag="sc")
                for h in range(2):
                    nc.tensor.matmul(
                        sc_t[:, h, :],
                        KT[64 * h:64 * (h + 1), p, kt * 128:(kt + 1) * 128],
                        QT[64 * h:64 * (h + 1), p, qc * 512:(qc + 1) * 512],
                        start=True, stop=True, tile_position=(64 * h, 0))
                nc.scalar.activation(et[:, kt], sc_t[:], AF.Exp)

        def new_et():
            return expp.tile([128, KT_TILES, 2, 512], BF16, tag="expT", name="et")

        def drain(g, n=10 ** 9):
            """Pull generator g up to n times; True if exhausted."""
            for _ in range(n):
                if next(g, StopIteration) is StopIteration:
                    return True
            return False

        # filler FIFO: entries (generator, est_us_per_yield); strict head-first
        # draining so generators holding PSUM tiles never interleave.
        fifo = []
        credit = [0.0]

        def pump(budget_us):
            credit[0] = min(credit[0] + budget_us, 2.5)
            while credit[0] > 0 and fifo:
                gen, cost = fifo[0]
                if drain(gen, 1):
                    fifo.pop(0)
                else:
                    credit[0] -= cost

        def fifo_drop(gen):
            for i, (g, _) in enumerate(fifo):
                if g is gen:
                    fifo.pop(i)
                    return

        def ensure(need):
            key, units = need
            gen = gqk_a if key == 'a' else gqk_b
            while prog[key] < units:
                if drain(gen, 1):
                    break

        # ---- emission ----
        # ACT table preload: dummy exp at t=0 so the ~2.7us exp table load
        # hides under the DMA lead-in (first real exp fires ~6us in).
        dummy = wpool.tile([1, 2], BF16, tag="dummy")
        nc.scalar.activation(dummy[:], ident[0:1, 0:2], AF.Exp)

        # weights on the Activation HWDGE ring; x inputs on the SP ring.
        nc.scalar.dma_start(wq_sb[:, 0], wq_ap[:, 0])
        nc.scalar.dma_start(wk_sb[:, 0], wk_ap[:, 0])
        nc.scalar.dma_start(wq_sb[:, 1], wq_ap[:, 1])
        nc.scalar.dma_start(wk_sb[:, 1], wk_ap[:, 1])
        nc.scalar.dma_start(bq_sb[:], bq_ap[:])
        nc.scalar.dma_start(bk_sb[:], bk_ap[:])
        nc.scalar.dma_start(wv_sb[:], wv_ap[:])
        nc.scalar.dma_start(bvf_sb[:], bvf_ap[:])
        nc.scalar.dma_start(wo_sb[:], wo_ap[:])

        # x DMA issue order: xq0 first (QT qc0 unlocks the backbone; fine
        # chunks spread it over more DMA queues for head latency), then xk
        # (consumed across all scs by every group head), then xv, then xq1-3.
        prefetch(xq_c, xq_ap, 0, chunks=4)
        prefetch(xk_c, xk_ap, 0, chunks=4)
        prefetch(xk_c, xk_ap, 1, chunks=2)
        prefetch(xk_c, xk_ap, 2)
        prefetch(xk_c, xk_ap, 3)

        # PE warmup (HAM ramp): a few matmuls on a zeroed tile during the
        # short DMA lead-in before xq0 lands.
        warm = wpool.tile([128, 256], BF16, tag="warm")
        nc.vector.memset(warm[:], 0.0)
        wps = ppo.tile([128, 512], F32, tag="ppo")
        for i in range(4):
            nc.tensor.matmul(wps[:, 0:256], warm[:, 0:128], warm[:],
                             start=(i == 0), stop=(i == 3))

        gqk_a = gen_projqk('a', QK_SEQ_A)
        gqk_b = gen_projqk('b', QK_SEQ_B)
        gv = gen_projv()

        ensure(need_qt(0, 0))             # QT pair-0 qc0
        fifo.append((gqk_a, 1.9))
        fifo.append((gv, 1.3))
        fifo.append((gqk_b, 1.9))

        # backbone: 8 groups of 16 kt slots; (pair, qc) interleaved. AVs are
        # APPENDED (never front-inserted): strict FIFO order guarantees the
        # V projection (Vones) is fully emitted before any AV reads it.
        order = [(0, 0), (1, 0), (0, 1), (1, 1), (0, 2), (1, 2), (0, 3), (1, 3)]
        ets = {}
        gas = {}
        gops = {}

        for gi, (p, qc) in enumerate(order):
            # group-entry guards
            ensure(need_qt(p, qc))               # QT (p, qc) projected
            if gi == 1:
                prefetch(xv_c, xv_ap, 0)
                prefetch(xv_c, xv_ap, 1)
                prefetch(xq_c, xq_ap, 1)
            if gi == 2:
                prefetch(xq_c, xq_ap, 2)
            if gi == 3:
                prefetch(xq_c, xq_ap, 3)
            # et buffer rotation (bufs=3): group gi reuses the buffer of
            # gi-3, whose AV must be fully drained first. Vones must be
            # complete before any AV pull (emission-order dataflow).
            if gi == 3:
                drain(gv)
            if gi >= 3:
                drain(gas[order[gi - 3]][0])
            if gi == 4:
                # out-proj st-group 0 needs ZT qc0 from both pairs
                # ((0,0) drained at gi=3, (1,0) = order[1] drained above)
                gops[0] = gen_outproj(range(0, 4))
                fifo.append((gops[0], 0.55))
            if gi == 6:
                # gop1 needs (0,1) [drained gi=5] and (1,1) = order[3] [above]
                gops[1] = gen_outproj(range(4, 8))
                fifo.append((gops[1], 0.55))
            if gi == 7:
                drain(gas[(1, 2)][0])
                gops[2] = gen_outproj(range(8, 12))
                fifo.append((gops[2], 0.55))

            et = ets.setdefault((p, qc), new_et())
            for kt in range(KT_TILES):
                ensure(need_kt(p, kt // 4))       # KT (p, sc) projected
                scores_kts(p, qc, et, [kt])
                pump(0.78)

            # AV for this group appended as filler (after gv in FIFO order)
            ga = gen_av(p, qc, et)
            gas[(p, qc)] = (ga, 0.33)
            fifo.append((ga, 0.33))

        # tail: av(0,3)/av(1,3) + the out-proj groups that depend on them
        ga13 = gas[(1, 3)][0]
        fifo_drop(ga13)
        drain(gv)          # ppo safety: gop3 allocs must not interleave gv's
        drain(gas[(0, 3)][0])
        drain(ga13, 19)    # h-loops + qt0/qt1 transposes
        gop3 = gen_outproj(range(12, 16), act_evict=True)
        drain(gop3, 4)     # st12, st13
        drain(ga13)        # qt2/qt3 transposes
        while fifo:
            if drain(fifo[0][0], 4):
                fifo.pop(0)
        drain(gop3)        # st14, st15

    nc.compile()
    return nc


def prep_inputs(query, key, value, Wq, bq, Wk, bk, Wv, bv, Wo, bo):
    """Host-side sharding: per-core input dicts (bf16, transposed/augmented)."""
    bf = ml_dtypes.bfloat16
    q32 = np.asarray(query, np.float32)
    k32 = np.asarray(key, np.float32)
    v32 = np.asarray(value, np.float32)
    Wq = np.asarray(Wq, np.float32)
    Wk = np.asarray(Wk, np.float32)
    Wv = np.asarray(Wv, np.float32)
    Wo = np.asarray(Wo, np.float32)
    bq = np.asarray(bq, np.float32)
    bk = np.asarray(bk, np.float32)
    bv = np.asarray(bv, np.float32)

    scale = 1.0 / np.sqrt(np.float32(D))

    def xt_layout(x2d):
        # [S, E] -> X^T [E, S] -> [sc, p, eo, j] contiguous tile layout
        a = x2d.T.reshape(ET, 128, QC, 512).transpose(2, 1, 0, 3)
        return np.ascontiguousarray(a).astype(bf)

    def w_layout(w2d):
        # [E, D'] -> [p, eo, D'] contiguous
        a = w2d.reshape(ET, 128, w2d.shape[1]).transpose(1, 0, 2)
        return np.ascontiguousarray(a).astype(bf)

    def w_layout_pair(w2d):
        # [E, 256] -> [p, pair, eo, 128] contiguous (pair-major so pair-0
        # loads alone in the head)
        a = w2d.reshape(ET, 128, 2, 128).transpose(1, 2, 0, 3)
        return np.ascontiguousarray(a).astype(bf)

    xt = {}
    for b in range(B):
        xt[('q', b)] = xt_layout(q32[b])
        xt[('k', b)] = xt_layout(k32[b])
        xt[('v', b)] = xt_layout(v32[b])

    in_maps = []
    for c in range(N_CORES):
        b, g = c // HL, c % HL
        hs = slice(g * DL, (g + 1) * DL)
        wv_aug = np.zeros((E, HL * 65), np.float32)
        bv_aug = np.zeros((1, HL * 65), np.float32)
        for h in range(HL):
            wv_aug[:, h * 65:h * 65 + D] = Wv[:, g * DL + h * D: g * DL + (h + 1) * D]
            bv_aug[0, h * 65:h * 65 + D] = bv[g * DL + h * D: g * DL + (h + 1) * D]
            bv_aug[0, h * 65 + D] = 1.0
        in_maps.append({
            "xq_t": xt[('q', b)],
            "xk_t": xt[('k', b)],
            "xv_t": xt[('v', b)],
            "wq": w_layout_pair(Wq[:, hs] * scale),
            "wk": w_layout_pair(Wk[:, hs]),
            "wv": w_layout(wv_aug),
            "bq": np.ascontiguousarray(
                (bq[hs] * scale).reshape(2, 128).T).astype(np.float32),
            "bk": np.ascontiguousarray(
                bk[hs].reshape(2, 128).T).astype(np.float32),
            "bvf": np.ascontiguousarray(
                np.broadcast_to(bv_aug, (128, HL * 65))).astype(bf),
            "wo": np.ascontiguousarray(
                Wo[hs, :].reshape(2, 128, E).transpose(1, 0, 2)).astype(bf),
        })
    return in_maps


_NC_CACHE = [None]


def get_nc():
    if _NC_CACHE[0] is None:
        _install_ntff_hook()
        _NC_CACHE[0] = build_kernel()
    return _NC_CACHE[0]


def run(inputs, trace=False):
    nc = get_nc()
    in_maps = prep_inputs(**{k: v for k, v in inputs.items() if k != 'bo'},
                          bo=inputs['bo'])
    res = bass_utils.run_bass_kernel_spmd(
        nc, in_maps, core_ids=list(range(N_CORES)), trace=trace)
    bo = np.asarray(inputs['bo'], np.float32)
    out = np.empty((B, S, E), np.float32)
    for b in range(B):
        acc = np.zeros((S, E), np.float32)
        for g in range(HL):
            acc += np.asarray(res.results[b * HL + g]["out_p"], np.float32)
        out[b] = acc + bo[None, :]
    return out, res


def kernel(**inputs):
    out, _ = run(inputs, trace=False)
    return out
